# revision 1
# baseline (speedup 1.0000x reference)
"""Trainium2 Bass kernel for AdaptiveGGNN TTE (8 NeuronCores, SPMD).

Strategy (pull-aggregation, no scatter):
  - Relabel nodes: trajectory nodes first, then per-core sort by padded
    in-degree J(d) (descending), constrained to preserve id%4 so that the
    residue class of every source is stable.  12500 real nodes per core,
    padded to 12544 rows; h is node-major fp16 [100352, 128], replicated
    per core via AllGather of the per-core slice.
  - Graph phase per step: edges grouped by OWNER core of dst (pull).  For
    each destination column (canonical order) and each residue q=src%4, its
    in-edges are gathered feature-major via dma_gather(transpose=True) from
    a fat-row view of h ([25088, 512] fp16, elem_step=512, byte offset
    q*256) so int16 indices cover all 100352 rows.  Every column is padded
    to J(d)=max(1, max_q deg_q(d)) slots per residue (pad slots read a
    guaranteed-zero row), so a single 4D strided tensor_reduce per run of
    equal-J columns does the whole segmented sum in fp32.  The result is
    cast to fp16 haggT chunks that feed the gated update directly — no
    scatter-add, no HBM accumulators.
  - GRU-style gated update runs feature-major on-chip (PE matmuls, ACT
    gates, DVE elementwise), transposed back to node-major and AllGathered.
  - Trajectory phase: 128 sequences (64 fwd + 64 time-reversed bwd) split
    16 per core; the embedding/bias/mask part of the input gates is folded
    on CPU into a dense GXE tensor (masking via a +40 override on the z
    gate); the h-dependent part and the 128-step recurrence run on-chip.
  - LayerNorm+GELU+FC head computed on every core; core 0's output used.
"""

import os
import sys

import numpy as np

sys.path.insert(0, "/opt/trn_rl_repo")

N = 100000
E = 1600000
F = 32
H = 128
IE = 32
DYN = 16
B = 64
L = 128
STEPS = 3
EPS = 1e-5

W = 8                     # cores
NPR = 12500               # real nodes per core
NPC = 12544               # padded rows per core slice (98 * 128)
NT = NPC // 128           # node tiles per core
NG = W * NPC              # padded global rows = 100352
R4 = NG // 4              # fat rows = 25088
CAP = 4608                # target gathered slots per (chunk, residue)
WMAX = 1024               # max chunk width (columns)
NSEQ = 16                 # sequences per core (128 total = 64 fwd + 64 bwd)


def _wrap_idx(a):
    """int16 index stream -> [128, n/16] wrapped layout (i -> [i%16, i//16]),
    replicated down the 128 partitions in groups of 16."""
    assert a.size % 16 == 0
    w = a.reshape(-1, 16).T.astype(np.int16)
    return np.tile(w, (8, 1)).copy()


def _preprocess(edge_index, x0, traj, lengths, dyn_feat, id_emb, W_proj, b_proj,
                Wz, bz, Wr, br, Wh, bh, Wih_f, Whh_f, bih_f, bhh_f,
                Wih_b, Whh_b, bih_b, bhh_b, ln_g, ln_b, fc1_W, fc1_b, fc2_W,
                fc2_b):
    edge_index = np.asarray(edge_index, np.int64)
    traj = np.asarray(traj, np.int64)
    lengths = np.asarray(lengths, np.int64)
    f32 = lambda a: np.asarray(a, np.float32)
    f16 = lambda a: np.ascontiguousarray(np.asarray(a, np.float32).astype(np.float16))
    x0, dyn_feat, id_emb = f32(x0), f32(dyn_feat), f32(id_emb)

    # ---- node relabeling pass 1: trajectory nodes first ----
    uniq = np.unique(traj)
    K = uniq.size
    old2new = np.full(N, -1, np.int64)
    old2new[uniq] = np.arange(K)
    rest = np.nonzero(old2new < 0)[0]
    old2new[rest] = K + np.arange(rest.size)

    src = old2new[edge_index[0]]
    dst = old2new[edge_index[1]]

    # padded-global row id of a node id (NPR and NPC are both %4==0, so
    # pad_gid(n) % 4 == n % 4 — the residue is determined by the id alone)
    def pad_gid(n):
        return (n // NPR) * NPC + (n % NPR)

    # ---- residue-class balancing ----
    # class(v) decides v's final id%4 (and so which fat-row byte-offset its
    # value is gathered with).  Balance each dst's in-edges across the 4
    # classes to minimize sum_d max_q deg_q(d) — the gather slot count.
    cls_of = np.arange(N) % 4                      # init: current id%4
    ar = np.arange(E)
    arn = np.arange(N)

    def _obj(cl):
        d = np.zeros((N, 4), np.int64)
        np.add.at(d, (dst, cl[src]), 1)
        return int(np.maximum(d.max(axis=1), 1).sum())

    # phase 1: quadratic potential (smooth, converges under parallel moves)
    for _p, rate in enumerate([0.5, 0.4, 0.3, 0.25, 0.2, 0.15, 0.12, 0.1]):
        deg_q = np.zeros((N, 4), np.int64)
        np.add.at(deg_q, (dst, cls_of[src]), 1)
        dq = deg_q[dst].astype(np.float64)
        a = cls_of[src]
        da = dq[ar, a]
        delta = 2.0 * (dq - da[:, None]) + 2.0
        delta[ar, a] = 0.0
        gain = np.stack([np.bincount(src, weights=delta[:, b], minlength=N)
                         for b in range(4)], axis=1)
        best_b = gain.argmin(axis=1)
        move = ((gain[arn, best_b] < 0)
                & (np.random.default_rng(_p).random(N) < rate))
        cls_of = np.where(move, best_b, cls_of)
    # phase 2: exact-max objective at low move rates, keep the best seen
    best_obj, best_cls = _obj(cls_of), cls_of.copy()
    for _p, rate in enumerate([0.2, 0.15, 0.12, 0.1, 0.08, 0.07, 0.06,
                               0.05, 0.05, 0.04, 0.04, 0.03, 0.03, 0.03,
                               0.02, 0.02, 0.02, 0.02, 0.015, 0.015,
                               0.01, 0.01, 0.01, 0.01]):
        deg_q = np.zeros((N, 4), np.int64)
        np.add.at(deg_q, (dst, cls_of[src]), 1)
        dq = deg_q[dst]
        a = cls_of[src]
        old_max = dq.max(axis=1)
        delta = np.empty((E, 4), np.float64)
        for b in range(4):
            t = dq.copy()
            t[ar, a] -= 1
            t[ar, b] += 1
            delta[:, b] = t.max(axis=1) - old_max
        gain = np.stack([np.bincount(src, weights=delta[:, b], minlength=N)
                         for b in range(4)], axis=1)
        best_b = gain.argmin(axis=1)
        move = ((gain[arn, best_b] < 0)
                & (np.random.default_rng(100 + _p).random(N) < rate))
        cls_of = np.where(move, best_b, cls_of)
        o = _obj(cls_of)
        if o < best_obj:
            best_obj, best_cls = o, cls_of.copy()
    cls_of = best_cls
    # enforce per-core class quotas (NPR/4 each) with least-harm moves
    deg_q = np.zeros((N, 4), np.int64)
    np.add.at(deg_q, (dst, cls_of[src]), 1)
    dq = deg_q[dst]
    a = cls_of[src]
    ar = np.arange(E)
    old_max = dq.max(axis=1)
    delta = np.empty((E, 4), np.int64)
    for b in range(4):
        t = dq.copy()
        t[ar, a] -= 1
        t[ar, b] += 1
        delta[:, b] = t.max(axis=1) - old_max
    gain = np.stack([np.bincount(src, weights=delta[:, b].astype(np.float64),
                                 minlength=N) for b in range(4)], axis=1)
    for c in range(W):
        ids = np.arange(c * NPR, (c + 1) * NPR)
        want = NPR // 4
        cnt = np.bincount(cls_of[ids], minlength=4)
        over = [r for r in range(4) if cnt[r] > want]
        under = [r for r in range(4) if cnt[r] < want]
        for r in over:
            surplus_ids = ids[cls_of[ids] == r]
            # cheapest candidates to move away from class r
            cost = gain[surplus_ids] - gain[surplus_ids, r][:, None]
            n_move = cnt[r] - want
            moved = 0
            order_s = np.argsort(cost.min(axis=1), kind="stable")
            for vi in order_s:
                if moved >= n_move:
                    break
                v = surplus_ids[vi]
                tgt = sorted(under, key=lambda u: cost[vi, u])
                for u in tgt:
                    if cnt[u] < want:
                        cls_of[v] = u
                        cnt[u] += 1
                        cnt[r] -= 1
                        moved += 1
                        break
        assert np.all(np.bincount(cls_of[ids], minlength=4) == want)

    deg_q = np.zeros((N, 4), np.int64)
    np.add.at(deg_q, (dst, cls_of[src]), 1)
    J_node = np.maximum(deg_q.max(axis=1), 1)      # per node (as dst)

    # ---- relabel pass 2: per-core sort by J desc; slot s%4 == class ----
    perm = np.empty(N, np.int64)                   # old-new-id -> final id
    for c in range(W):
        base = c * NPR
        ids = np.arange(base, base + NPR)
        order = []
        for r in range(4):
            cls = ids[cls_of[ids] == r]
            cls = cls[np.argsort(-J_node[cls], kind="stable")]
            order.append(cls)
        # slot s (residue s%4) takes the next node of that residue class
        out = np.empty(NPR, np.int64)
        ptr = [0, 0, 0, 0]
        for s in range(NPR):
            r = s % 4
            out[s] = order[r][ptr[r]]
            ptr[r] += 1
        perm[out] = ids                            # node out[s] -> id base+s
    old2new = perm[old2new]
    src = perm[src]
    dst = perm[dst]
    traj_new = old2new[traj]                       # final ids of traj nodes
    new2old = np.empty(N, np.int64)
    new2old[old2new] = np.arange(N)

    src_g = pad_gid(src)
    assert np.all(src_g % 4 == src % 4)
    owner = dst // NPR
    dst_loc = dst % NPR

    # ---- final per-column J per core, shared program J ----
    deg_qc = np.zeros((W, NPC, 4), np.int64)
    np.add.at(deg_qc, (owner, dst_loc, src_g % 4), 1)
    J_core = np.maximum(deg_qc.max(axis=2), 1)     # [W, NPC]
    J_core[:, NPR:] = 0                            # pad columns: no slots
    J_prog = J_core.max(axis=0)                    # [NPC] shared structure
    # J_prog should be ~descending; runs come from its actual values.

    # ---- chunks: widths %128, <=1024 cols, sum(J_prog) <= CAP ----
    chunks = []                                    # (col0, width, S_pad, runs)
    col = 0
    while col < NPC:
        width = 0
        sj = 0
        while width < WMAX and col + width < NPC:
            g = min(128, NPC - col - width)
            gj = int(J_prog[col + width:col + width + g].sum())
            if width and sj + gj > CAP:
                break
            sj += gj
            width += g
        assert width % 128 == 0 and width > 0
        S_pad = ((sj + 127) // 128) * 128
        if S_pad == 0:
            S_pad = 128                            # all-pad chunk (tail)
        assert S_pad <= 8064, (col, width, S_pad)
        # runs of equal J (J>0) within the chunk
        runs = []
        j_prev = -1
        for d in range(width):
            j = int(J_prog[col + d])
            if j == 0:
                break
            if j == j_prev:
                runs[-1][1] += 1
            else:
                runs.append([d, 1, j])
                j_prev = j
        run_list = []
        off = 0
        for d0, D, j in runs:
            run_list.append((off, d0, D, j))
            off += D * j
        chunks.append((col, width, S_pad, tuple(run_list)))
        col += width
    S_MAX = max(c[2] for c in chunks)
    TOT = sum(4 * c[2] for c in chunks)

    # ---- per-core gather index streams ----
    # order edges by (owner, dst_loc, residue); within a (col, q) any order
    o = np.lexsort((src_g, src_g % 4, dst_loc, owner))
    e_owner = owner[o]
    e_dloc = dst_loc[o]
    e_q = (src_g % 4)[o]
    e_idx = (src_g // 4)[o]
    g4 = np.empty((W, TOT), np.int16)
    for c in range(W):
        pad_idx = c * (NPC // 4) + (NPR // 4) + 3  # row c*NPC+12512 (+q) == 0
        m = e_owner == c
        dl, qq, ii = e_dloc[m], e_q[m], e_idx[m]
        # slot start per (col, q): cumulative J_prog per column
        cnt = np.zeros((NPC, 4), np.int64)
        np.add.at(cnt, (dl, qq), 1)
        assert np.all(cnt.max(axis=1) <= np.maximum(J_prog, 1))
        stream = np.full(TOT, pad_idx, np.int64)
        base = 0
        # j-major run layout: position of (col d, residue q, rank j) =
        # sec_base[chunk(d), q] + run_off[d] + j * D_run[d] + (d - d0_run[d])
        # so that consecutive descriptors sweep the j-th-smallest sources of
        # a whole run — a narrow band of HBM rows (row-buffer locality).
        sec_base = np.zeros((len(chunks), 4), np.int64)
        run_off = np.zeros(NPC, np.int64)
        D_col = np.ones(NPC, np.int64)
        d0_col = np.zeros(NPC, np.int64)
        for ci, (col0, width, S_pad, run_list) in enumerate(chunks):
            for (off, d0, D, j) in run_list:
                run_off[col0 + d0: col0 + d0 + D] = off
                D_col[col0 + d0: col0 + d0 + D] = D
                d0_col[col0 + d0: col0 + d0 + D] = col0 + d0
            for q in range(4):
                sec_base[ci, q] = base
                base += S_pad
        assert base == TOT
        chunk_of_col = np.zeros(NPC, np.int64)
        for ci, (col0, width, _S, _r) in enumerate(chunks):
            chunk_of_col[col0:col0 + width] = ci
        order2 = np.lexsort((ii, qq, dl))
        dl2, qq2, ii2 = dl[order2], qq[order2], ii[order2]
        key = dl2 * 4 + qq2
        grp_start = np.r_[True, key[1:] != key[:-1]]
        first = np.maximum.accumulate(np.where(grp_start, np.arange(key.size), 0))
        rank = np.arange(key.size) - first
        pos = (sec_base[chunk_of_col[dl2], qq2] + run_off[dl2]
               + rank * D_col[dl2] + (dl2 - d0_col[dl2]))
        assert np.all(rank < J_prog[dl2])
        stream[pos] = ii2
        g4[c] = stream.astype(np.int16)

    # ---- per-core x0 (feature-major, padded) ----
    x0_new = x0[new2old]                           # [N, F]
    x0T = np.zeros((W, F, NPC), np.float16)
    for c in range(W):
        x0T[c, :, :NPR] = x0_new[c * NPR:(c + 1) * NPR].T.astype(np.float16)

    # ---- trajectory phase ----
    emb = id_emb.copy()
    emb[0] = 0.0                                    # padding_idx on ORIGINAL id 0
    emb_seq = emb[traj]                             # [B, L, IE]
    Wih = {0: np.asarray(Wih_f, np.float32), 1: np.asarray(Wih_b, np.float32)}
    bih = {0: np.asarray(bih_f, np.float32), 1: np.asarray(bih_b, np.float32)}
    bhh = {0: np.asarray(bhh_f, np.float32), 1: np.asarray(bhh_b, np.float32)}
    Whh = {0: np.asarray(Whh_f, np.float32), 1: np.asarray(Whh_b, np.float32)}

    tg_idx = np.zeros((W, L * NSEQ), np.int16)      # gather idx, col = t*16+s
    GXE = np.zeros((W, H, 3, L * NSEQ), np.float32)
    WhhT = np.zeros((W, 3, H, H), np.float16)       # lhsT per gate (r,z,n)
    WihT = np.zeros((W, 3, H, H), np.float16)
    for c in range(W):
        d = c // 4                                  # 0 fwd, 1 bwd
        bs = (c % 4) * NSEQ + np.arange(NSEQ)       # sample ids
        t_eff = np.arange(L) if d == 0 else (L - 1 - np.arange(L))
        nodes = traj_new[bs][:, t_eff]              # [NSEQ, L]
        tg_idx[c] = nodes.T.reshape(-1).astype(np.int16)   # col = t*16+s
        e = emb_seq[bs][:, t_eff]                   # [NSEQ, L, IE]
        for g in range(3):
            Wg_e = Wih[d][g * H:(g + 1) * H, H:]    # [H, IE]
            gx = np.einsum("hi,sti->hst", Wg_e, e).reshape(H, NSEQ * L)
            gx = gx.reshape(H, NSEQ, L).transpose(0, 2, 1).reshape(H, L * NSEQ)
            gx += (bih[d][g * H:(g + 1) * H] +
                   (bhh[d][g * H:(g + 1) * H] if g < 2 else 0.0))[:, None]
            GXE[c, :, g, :] = gx
            WhhT[c, g] = Whh[d][g * H:(g + 1) * H, :].T.astype(np.float16)
            WihT[c, g] = Wih[d][g * H:(g + 1) * H, :H].T.astype(np.float16)
        mask_pad = t_eff[None, :] >= lengths[bs][:, None]   # [NSEQ, L]
        padcols = np.nonzero(mask_pad.T.reshape(-1))[0]
        GXE[c, :, 1, padcols] = 40.0
    assert not np.any(bhh[0][2 * H:]) and not np.any(bhh[1][2 * H:]), \
        "nonzero bhh_n not folded (unsupported fast path)"

    # ---- head constants ----
    ln_g, ln_b = f32(ln_g), f32(ln_b)
    fc1_W, fc1_b = f32(fc1_W), f32(fc1_b)
    fc2_W, fc2_b = f32(fc2_W), f32(fc2_b)
    W1g = ln_g[:, None] * fc1_W[:2 * H]             # [256, H]
    W1a = np.ascontiguousarray(W1g[:H])
    W1b = np.ascontiguousarray(W1g[H:])
    W1d = np.ascontiguousarray(fc1_W[2 * H:])       # [16, 128]
    c1 = (ln_b @ fc1_W[:2 * H] + fc1_b).reshape(H, 1)
    dynT = np.ascontiguousarray(dyn_feat.T)         # [16, 64]
    w2 = np.ascontiguousarray(fc2_W.reshape(H, 1))
    b2 = float(np.asarray(fc2_b).reshape(-1)[0])

    Wzf, Wrf, Whf = f32(Wz), f32(Wr), f32(Wh)

    plan = dict(chunks=chunks, S_MAX=S_MAX, TOT=TOT, b2=b2, gelu_exact=True)
    shared = dict(
        wproj=f16(np.asarray(W_proj, np.float32)),          # [32,128] lhsT
        bproj=f32(b_proj).reshape(H, 1),
        # z/r gates use sigmoid-via-tanh: biases pre-halved on CPU
        wz_h=f16(Wzf[F:]), wz_x=f16(Wzf[:F]), bz=0.5 * f32(bz).reshape(H, 1),
        wr_h=f16(Wrf[F:]), wr_x=f16(Wrf[:F]), br=0.5 * f32(br).reshape(H, 1),
        wh_h=f16(Whf[F:]), wh_x=f16(Whf[:F]), bh=f32(bh).reshape(H, 1),
        w1a=W1a.astype(np.float32), w1b=W1b.astype(np.float32),
        w1d=W1d.astype(np.float32), c1=c1.astype(np.float32),
        dynT=dynT.astype(np.float32), w2=w2.astype(np.float32),
        id16=np.eye(128, dtype=np.float16), id32=np.eye(128, dtype=np.float32),
    )
    in_maps = []
    for c in range(W):
        m = dict(shared)
        m["g4"] = _wrap_idx(g4[c])
        m["x0T"] = x0T[c]
        m["tg_idx"] = _wrap_idx(tg_idx[c])
        m["gxe"] = GXE[c]
        m["whh_r"], m["whh_z"], m["whh_n"] = WhhT[c, 0], WhhT[c, 1], WhhT[c, 2]
        m["wih_r"], m["wih_z"], m["wih_n"] = WihT[c, 0], WihT[c, 1], WihT[c, 2]
        in_maps.append(m)
    extras = dict(b2=b2)
    return in_maps, plan, extras


# ---------------------------------------------------------------------------
# numpy emulation of the device program (for fast logic validation)
# ---------------------------------------------------------------------------

def _emulate(in_maps, plan, extras):
    chunks = plan["chunks"]

    def unwrap(w):
        return w[:16].T.reshape(-1).astype(np.int64)

    h_full = np.zeros((NG, H), np.float16)
    h_ownT = {}
    for c in range(W):
        m = in_maps[c]
        pre = m["wproj"].astype(np.float32).T @ m["x0T"].astype(np.float32)
        h0T = np.tanh(pre + m["bproj"])
        h0T[:, NPR:] = 0.0
        h_ownT[c] = h0T
        h_full[c * NPC:(c + 1) * NPC] = h0T.T.astype(np.float16)

    S_MAX = plan["S_MAX"]
    for step in range(STEPS):
        newf = np.zeros_like(h_full)
        for c in range(W):
            m = in_maps[c]
            g = unwrap(m["g4"])
            haggT = np.zeros((H, NPC), np.float32)
            base = 0
            for (col0, width, S_pad, runs) in chunks:
                # emulate the 4 transpose-gathers + 4D reduce
                vals = np.zeros((H, 4, S_pad), np.float32)
                for q in range(4):
                    idx = g[base + q * S_pad: base + (q + 1) * S_pad]
                    vals[:, q, :] = h_full[4 * idx + q].astype(np.float32).T
                for (off, d0, D, j) in runs:
                    v = vals[:, :, off:off + D * j].reshape(H, 4, j, D)
                    haggT[:, col0 + d0: col0 + d0 + D] = v.sum(axis=(1, 2))
                base += 4 * S_pad
            hagg16 = haggT.astype(np.float16).astype(np.float32)
            x0T = m["x0T"].astype(np.float32)
            # z/r via tanh form with pre-halved biases (matches device)
            z = 0.5 * np.tanh(0.5 * (m["wz_h"].astype(np.float32).T @ hagg16 +
                                     m["wz_x"].astype(np.float32).T @ x0T)
                              + m["bz"]) + 0.5
            r = 0.5 * np.tanh(0.5 * (m["wr_h"].astype(np.float32).T @ hagg16 +
                                     m["wr_x"].astype(np.float32).T @ x0T)
                              + m["br"]) + 0.5
            ht = np.tanh(m["wh_h"].astype(np.float32).T @ (r * hagg16) +
                         m["wh_x"].astype(np.float32).T @ x0T + m["bh"])
            hn = hagg16 + z * (ht - hagg16)
            hn[:, NPR:] = 0.0
            newf[c * NPC:(c + 1) * NPC] = hn.T.astype(np.float16)
        h_full = newf

    # trajectory
    states = np.zeros((128, H), np.float32)
    for c in range(W):
        m = in_maps[c]
        tg = unwrap(m["tg_idx"])[:L * NSEQ]
        hT = h_full[tg].astype(np.float32).T       # [H, L*NSEQ]
        GX = np.empty((H, 3, L * NSEQ), np.float32)
        for gt, key in enumerate(["wih_r", "wih_z", "wih_n"]):
            GX[:, gt, :] = (m[key].astype(np.float32).T @ hT +
                            m["gxe"][:, gt, :])
        h32 = np.zeros((H, NSEQ), np.float32)
        for t in range(L):
            h16 = h32.astype(np.float16).astype(np.float32)
            ghr = m["whh_r"].astype(np.float32).T @ h16
            ghz = m["whh_z"].astype(np.float32).T @ h16
            ghn = m["whh_n"].astype(np.float32).T @ h16
            sl = slice(t * NSEQ, (t + 1) * NSEQ)
            r = 1 / (1 + np.exp(-(ghr + GX[:, 0, sl])))
            z = 1 / (1 + np.exp(-(ghz + GX[:, 1, sl])))
            n = np.tanh(r * ghn + GX[:, 2, sl])
            h32 = n + z * (h32 - n)
        states[c * NSEQ:(c + 1) * NSEQ] = h32.T
    # head
    m = in_maps[0]
    ST = states.T
    S1, S2 = ST[:, :64], ST[:, 64:]
    mu = (S1.sum(0) + S2.sum(0)) / 256.0
    Sc1, Sc2 = S1 - mu, S2 - mu
    ssq = (Sc1 ** 2).sum(0) + (Sc2 ** 2).sum(0)
    rstd = 1.0 / np.sqrt(ssq / 256.0 + EPS)
    P = m["w1a"].T @ Sc1 + m["w1b"].T @ Sc2
    t2 = P * rstd[None, :] + m["w1d"].T @ m["dynT"] + m["c1"]
    if plan.get("gelu_exact", True):
        from scipy.special import erf
        z1 = 0.5 * t2 * (1.0 + erf(t2 / np.sqrt(2.0)))
    else:
        z1 = t2 * (1.0 / (1.0 + np.exp(-1.702 * t2)))
    out = z1.T @ m["w2"][:, 0] + extras["b2"]
    return out.astype(np.float32)


# ---------------------------------------------------------------------------
# Bass program
# ---------------------------------------------------------------------------

def _build(plan):
    import concourse.bass as bass
    import concourse.bacc as bacc
    import concourse.mybir as mybir
    import concourse.tile as tile

    dt = mybir.dt
    AF = mybir.ActivationFunctionType
    AL = mybir.AluOpType
    AX = mybir.AxisListType
    chunks = plan["chunks"]
    S_MAX = plan["S_MAX"]
    TOT = plan["TOT"]
    WMAXc = max(c[1] for c in chunks)
    b2c = float(plan["b2"])
    dbg = plan.get("dbg", {})
    n_steps = dbg.get("steps", STEPS)
    do_traj = dbg.get("traj", True)
    do_head = dbg.get("head", True)
    chunk_lim = dbg.get("segs_limit", None)
    chunks_used = chunks if chunk_lim is None else chunks[:chunk_lim]
    skip_gather = dbg.get("skip_gather", False)
    skip_reduce = dbg.get("skip_reduce", False)
    skip_upd = dbg.get("skip_upd", False)
    tsteps = dbg.get("tsteps", L)
    notables = dbg.get("notables", False)
    no_ag = dbg.get("no_ag", False)
    empty = dbg.get("empty", False)

    nc = bacc.Bacc(None, target_bir_lowering=False, debug=False, num_devices=W,
                   dynamic_dma_scratch_size=32768)
    AF_SIG = AF.Identity if notables else AF.Sigmoid
    AF_TANH = AF.Identity if notables else AF.Tanh
    di = lambda nm, shp, d: nc.dram_tensor(nm, shp, d, kind="ExternalInput")

    g4_d = di("g4", [128, TOT // 16], dt.int16)
    x0T_d = di("x0T", [F, NPC], dt.float16)
    tg_idx = di("tg_idx", [128, L * NSEQ // 16], dt.int16)
    gxe_d = di("gxe", [H, 3, L * NSEQ], dt.float32)
    wproj = di("wproj", [F, H], dt.float16)
    bproj = di("bproj", [H, 1], dt.float32)
    gate_w = {}
    for gname in ("z", "r", "h"):
        gate_w[gname] = (
            di(f"w{gname}_h", [H, H], dt.float16),
            di(f"w{gname}_x", [F, H], dt.float16),
            di(f"b{gname}", [H, 1], dt.float32),
        )
    whh = {g: di(f"whh_{g}", [H, H], dt.float16) for g in ("r", "z", "n")}
    wih = {g: di(f"wih_{g}", [H, H], dt.float16) for g in ("r", "z", "n")}
    w1a = di("w1a", [H, H], dt.float32)
    w1b = di("w1b", [H, H], dt.float32)
    w1d = di("w1d", [DYN, H], dt.float32)
    c1_d = di("c1", [H, 1], dt.float32)
    dynT_d = di("dynT", [DYN, B], dt.float32)
    w2_d = di("w2", [H, 1], dt.float32)
    id16_d = di("id16", [128, 128], dt.float16)
    id32_d = di("id32", [128, 128], dt.float32)
    out_d = nc.dram_tensor("out", [B, 1], dt.float32, kind="ExternalOutput")

    h_own = nc.dram_tensor("h_own", [NPC, H], dt.float16, kind="Internal")
    h_full = [nc.dram_tensor(f"h_full{k}", [NG, H], dt.float16, kind="Internal",
                             addr_space="Shared") for k in range(STEPS + 1)]
    ag_in = nc.dram_tensor("ag_in", [NSEQ, H], dt.float32, kind="Internal")
    ag_out = nc.dram_tensor("ag_out", [W * NSEQ, H], dt.float32,
                            kind="Internal", addr_space="Shared")
    RG = [list(range(W))]
    h_own_v = h_own.rearrange("(t p) f -> p t f", p=128)   # [128, NT, H]

    if empty:
        with tile.TileContext(nc) as tc:
            with tc.tile_pool(name="dummy0", bufs=1) as dp:
                dz = dp.tile([B, 1], dt.float32)
                nc.vector.memset(dz[:], 0.5)
                nc.sync.dma_start(out_d[:], dz[:])
        nc.compile()
        return nc

    with tile.TileContext(nc) as tc:
        with tc.tile_pool(name="persist", bufs=1) as pp:
            id16 = pp.tile([128, 128], dt.float16)
            nc.sync.dma_start(id16[:], id16_d[:])
            wproj_t = pp.tile([F, H], dt.float16)
            nc.sync.dma_start(wproj_t[:], wproj[:])
            bproj_t = pp.tile([H, 1], dt.float32)
            nc.sync.dma_start(bproj_t[:], bproj[:])
            half05 = pp.tile([H, 1], dt.float32, tag="half05", name="half05")
            nc.vector.memset(half05[:], 0.5)
            gw = {}
            for gname in ("z", "r", "h"):
                wh_d, wx_d, b_d = gate_w[gname]
                wh_t = pp.tile([H, H], dt.float16, tag=f"w{gname}h",
                               name=f"wh_t_{gname}")
                wx_t = pp.tile([F, H], dt.float16, tag=f"w{gname}x",
                               name=f"wx_t_{gname}")
                b_t = pp.tile([H, 1], dt.float32, tag=f"b{gname}",
                              name=f"b_t_{gname}")
                nc.sync.dma_start(wh_t[:], wh_d[:])
                nc.sync.dma_start(wx_t[:], wx_d[:])
                nc.sync.dma_start(b_t[:], b_d[:])
                gw[gname] = (wh_t, wx_t, b_t)
            # per-chunk haggT tiles (persist across the step)
            hagg_t = [pp.tile([H, chunks[ci][1]], dt.float16, tag=f"hagg{ci}",
                              name=f"hagg{ci}")
                      for ci in range(len(chunks))]

            def emit_update_blk(hTb, col_base, blk, out_nm, ctx_pools, first):
                """One gated-update block: hTb fp16 [128, blk] feature-major
                (None for first), writes node-major fp16 into out_nm via PE
                transpose.  Pad columns (>= NPR) are forced to zero so they
                can serve as zero-sources for gather padding."""
                up, ups = ctx_pools
                pad_from = NPR - col_base if col_base + blk > NPR else None
                x0c_t = up.tile([F, 512], dt.float16, tag="x0c", name="x0c")
                nc.sync.dma_start(x0c_t[:, :blk],
                                  x0T_d[:, col_base:col_base + blk])
                x0c = x0c_t[:, :blk]
                if first:
                    hn = up.tile([H, 512], dt.float16, tag="hn", name="hn")
                    ps = ups.tile([H, 512], dt.float32, tag="psg", bufs=6,
                                  name="psg")
                    nc.tensor.matmul(ps[:, :blk], wproj_t[:], x0c)
                    nc.scalar.activation(hn[:, :blk], ps[:, :blk], AF.Tanh,
                                         bias=bproj_t[:])
                else:
                    ps = ups.tile([H, 512], dt.float32, tag="psg", bufs=6,
                                  name="psg")
                    nc.tensor.matmul(ps[:, :blk], gw["z"][0][:], hTb,
                                     start=True, stop=False)
                    nc.tensor.matmul(ps[:, :blk], gw["z"][1][:], x0c,
                                     start=False, stop=True)
                    zraw = up.tile([H, 512], dt.float16, tag="zraw", name="zraw")
                    nc.scalar.activation(zraw[:, :blk], ps[:, :blk],
                                         AF.Tanh, scale=0.5, bias=gw["z"][2][:])
                    z16 = up.tile([H, 512], dt.float16, tag="z16", name="z16")
                    nc.scalar.activation(z16[:, :blk], zraw[:, :blk],
                                         AF.Identity, scale=0.5, bias=half05[:])
                    ps2 = ups.tile([H, 512], dt.float32, tag="psg", bufs=6,
                                   name="psg2")
                    nc.tensor.matmul(ps2[:, :blk], gw["r"][0][:], hTb,
                                     start=True, stop=False)
                    nc.tensor.matmul(ps2[:, :blk], gw["r"][1][:], x0c,
                                     start=False, stop=True)
                    rraw = up.tile([H, 512], dt.float16, tag="rraw", name="rraw")
                    nc.scalar.activation(rraw[:, :blk], ps2[:, :blk],
                                         AF.Tanh, scale=0.5, bias=gw["r"][2][:])
                    r16 = up.tile([H, 512], dt.float16, tag="r16", name="r16")
                    nc.scalar.activation(r16[:, :blk], rraw[:, :blk],
                                         AF.Identity, scale=0.5, bias=half05[:])
                    rh = up.tile([H, 512], dt.float16, tag="rh", name="rh")
                    nc.vector.tensor_mul(rh[:, :blk], r16[:, :blk], hTb)
                    ps3 = ups.tile([H, 512], dt.float32, tag="psg", bufs=6,
                                   name="psg3")
                    nc.tensor.matmul(ps3[:, :blk], gw["h"][0][:],
                                     rh[:, :blk], start=True, stop=False)
                    nc.tensor.matmul(ps3[:, :blk], gw["h"][1][:], x0c,
                                     start=False, stop=True)
                    ht = up.tile([H, 512], dt.float16, tag="ht", name="ht")
                    nc.scalar.activation(ht[:, :blk], ps3[:, :blk], AF.Tanh,
                                         bias=gw["h"][2][:])
                    d16 = up.tile([H, 512], dt.float16, tag="d16", name="d16")
                    nc.vector.tensor_sub(d16[:, :blk], ht[:, :blk], hTb)
                    zd = up.tile([H, 512], dt.float16, tag="zd", name="zd")
                    nc.vector.tensor_mul(zd[:, :blk], z16[:, :blk],
                                         d16[:, :blk])
                    hn = up.tile([H, 512], dt.float16, tag="hn", name="hn")
                    nc.vector.tensor_add(hn[:, :blk], hTb, zd[:, :blk])
                if pad_from is not None:
                    nc.vector.memset(hn[:, pad_from:blk], 0.0)
                for q in range(blk // 128):
                    tp_ps = ups.tile([128, 128], dt.float16, tag="tps",
                                     name="tp_ps")
                    nc.tensor.transpose(tp_ps[:],
                                        hn[:, q * 128:(q + 1) * 128],
                                        id16[:])
                    ti = col_base // 128 + q
                    nc.vector.tensor_copy(out_nm[:, ti, :], tp_ps[:])
                # stream this block's node-major tiles to HBM immediately so
                # the write overlaps later chunks instead of the step tail
                t0 = col_base // 128
                nc.sync.dma_start(h_own_v[:, t0:t0 + blk // 128, :],
                                  out_nm[:, t0:t0 + blk // 128, :])

            # ---- h0 = tanh(W_proj^T x0) ----
            with (
                tc.tile_pool(name="h0", bufs=3) as hp,
                tc.tile_pool(name="h0ps", bufs=2, space="PSUM") as hps0,
            ):
                h_nm = hp.tile([128, NT, H], dt.float16, tag="h_nm", bufs=1)
                for jb in range(0, NPC, 512):
                    blk = min(512, NPC - jb)
                    emit_update_blk(None, jb, blk, h_nm, (hp, hps0),
                                    first=True)
            if not no_ag:
                nc.gpsimd.collective_compute(
                    "AllGather", mybir.AluOpType.bypass, replica_groups=RG,
                    ins=[h_own[:]], outs=[h_full[0][:]])

            # ---- message-passing steps ----
            for step in range(n_steps):
                hf = h_full[step]
                hf4 = hf.rearrange("(r four) f -> r (four f)", four=4)
                with (
                    tc.tile_pool(name=f"gs{step}", bufs=2) as gp,
                    tc.tile_pool(name=f"upd{step}", bufs=3) as up,
                    tc.tile_pool(name=f"updps{step}", bufs=2,
                                 space="PSUM") as ups,
                ):
                    h_nm = up.tile([128, NT, H], dt.float16, tag="h_nm",
                                   bufs=1, name="h_nm")
                    off16 = 0
                    for ci, (col0, width, S_pad, runs) in enumerate(chunks_used):
                        gi = gp.tile([128, 4 * S_MAX // 16], dt.int16,
                                     tag="gi", name="gi")
                        nc.sync.dma_start(
                            gi[:, :4 * S_pad // 16],
                            g4_d[:, off16:off16 + 4 * S_pad // 16])
                        vals = gp.tile([128, 4, S_MAX], dt.float16,
                                       tag="vals", name="vals")
                        if not skip_gather:
                            for q in range(4):
                                nc.gpsimd.dma_gather(
                                    vals[:, q:q + 1, :S_pad],
                                    hf4[:, q * 128:(q + 1) * 128],
                                    gi[:, q * S_pad // 16:(q + 1) * S_pad // 16],
                                    S_pad, S_pad, H, elem_step=512,
                                    transpose=True, single_packet=False)
                        else:
                            nc.vector.memset(vals[:], 0.0)
                        acc = gp.tile([128, WMAXc], dt.float32, tag="acc",
                                      name="acc")
                        if not skip_reduce:
                            cov = 0
                            for (off, d0, D, j) in runs:
                                nc.vector.tensor_reduce(
                                    acc[:, d0:d0 + D],
                                    vals[:, :, off:off + D * j].rearrange(
                                        "p q (j d) -> p d q j", j=j),
                                    AX.XY, AL.add)
                                cov = d0 + D
                            if cov < width:
                                nc.vector.memset(acc[:, cov:width], 0.0)
                        else:
                            nc.vector.memset(acc[:, :width], 0.0)
                        nc.vector.tensor_copy(hagg_t[ci][:],
                                              acc[:, :width])
                        off16 += 4 * S_pad // 16
                        if not skip_upd:
                            for sub in range(0, width, 512):
                                blk = min(512, width - sub)
                                emit_update_blk(
                                    hagg_t[ci][:, sub:sub + blk],
                                    col0 + sub, blk, h_nm, (up, ups),
                                    first=False)
                    if skip_upd:
                        continue
                if not no_ag:
                    nc.gpsimd.collective_compute(
                        "AllGather", mybir.AluOpType.bypass, replica_groups=RG,
                        ins=[h_own[:]], outs=[h_full[step + 1][:]])

            # ---- trajectory phase ----
            if not do_traj:
                with tc.tile_pool(name="dummy", bufs=1) as dp:
                    dz = dp.tile([B, 1], dt.float32)
                    nc.vector.memset(dz[:], 0.5)
                    nc.sync.dma_start(out_d[:], dz[:])
            if do_traj:
              with (
                tc.tile_pool(name="traj", bufs=1) as tp,
                tc.tile_pool(name="trajh", bufs=2) as th,
                tc.tile_pool(name="trajps", bufs=2, space="PSUM") as tps,
                tc.tile_pool(name="recpsp", bufs=3, space="PSUM") as rps,
            ):
                tgi = tp.tile([128, L * NSEQ // 16], dt.int16)
                nc.sync.dma_start(tgi[:], tg_idx[:])
                tv = tp.tile([H, 1, L * NSEQ], dt.float16)
                nc.gpsimd.dma_gather(tv[:], h_full[STEPS][0:NPC, :], tgi[:],
                                     L * NSEQ, L * NSEQ, H, transpose=True,
                                     single_packet=False)
                gxe_t = tp.tile([H, 3, L * NSEQ], dt.float32)
                nc.sync.dma_start(gxe_t[:], gxe_d[:])
                wih_t = {}
                whh_t = {}
                for g in ("r", "z", "n"):
                    wih_t[g] = tp.tile([H, H], dt.float16, tag=f"wih{g}",
                                       name=f"wih_t_{g}")
                    nc.sync.dma_start(wih_t[g][:], wih[g][:])
                    whh_t[g] = tp.tile([H, H], dt.float16, tag=f"whh{g}",
                                       name=f"whh_t_{g}")
                    nc.sync.dma_start(whh_t[g][:], whh[g][:])
                GX = tp.tile([H, 3, L * NSEQ], dt.float32)
                for gidx, g in enumerate(("r", "z", "n")):
                    for jb in range(0, L * NSEQ, 512):
                        ps = tps.tile([H, 512], dt.float32, tag="gxps",
                                      name="gxps")
                        nc.tensor.matmul(ps[:], wih_t[g][:],
                                         tv[:, 0, jb:jb + 512])
                        nc.vector.tensor_add(GX[:, gidx, jb:jb + 512], ps[:],
                                             gxe_t[:, gidx, jb:jb + 512])
                h16 = th.tile([H, NSEQ], dt.float16, tag="h16", name="h16")
                h32 = th.tile([H, NSEQ], dt.float32, tag="h32", name="h32")
                nc.vector.memset(h16[:], 0.0)
                nc.vector.memset(h32[:], 0.0)
                for t in range(tsteps):
                    sl = slice(t * NSEQ, (t + 1) * NSEQ)
                    ps = rps.tile([H, 3 * NSEQ], dt.float32, tag="recps",
                                  name="recps")
                    nc.tensor.matmul(ps[:, 0:NSEQ], whh_t["r"][:], h16[:])
                    nc.tensor.matmul(ps[:, NSEQ:2 * NSEQ], whh_t["z"][:], h16[:])
                    nc.tensor.matmul(ps[:, 2 * NSEQ:], whh_t["n"][:], h16[:])
                    rz_pre = th.tile([H, 2 * NSEQ], dt.float32, tag="rzpre",
                                     name="rz_pre")
                    nc.vector.tensor_add(
                        rz_pre[:].rearrange("p (g s) -> p g s", g=2),
                        ps[:, 0:2 * NSEQ].rearrange("p (g s) -> p g s", g=2),
                        GX[:, 0:2, sl])
                    rzt = th.tile([H, 2 * NSEQ], dt.float32, tag="rzt",
                                  name="rzt")
                    nc.scalar.activation(rzt[:], rz_pre[:], AF_TANH, scale=0.5)
                    rz = th.tile([H, 2 * NSEQ], dt.float32, tag="rz", name="rz")
                    nc.scalar.activation(rz[:], rzt[:], AF.Identity, scale=0.5,
                                         bias=half05[:])
                    nm1 = th.tile([H, NSEQ], dt.float32, tag="nm1", name="nm1")
                    nc.vector.tensor_mul(nm1[:], rz[:, 0:NSEQ],
                                         ps[:, 2 * NSEQ:])
                    nm2 = th.tile([H, NSEQ], dt.float32, tag="nm2", name="nm2")
                    nc.vector.tensor_add(nm2[:], nm1[:], GX[:, 2, sl])
                    nt_ = th.tile([H, NSEQ], dt.float32, tag="nt", name="nt_")
                    nc.scalar.activation(nt_[:], nm2[:], AF_TANH)
                    dd = th.tile([H, NSEQ], dt.float32, tag="dd", name="dd")
                    nc.vector.tensor_sub(dd[:], h32[:], nt_[:])
                    zd2 = th.tile([H, NSEQ], dt.float32, tag="zd2", name="zd2")
                    nc.vector.tensor_mul(zd2[:], rz[:, NSEQ:], dd[:])
                    h32 = th.tile([H, NSEQ], dt.float32, tag="h32", name="h32")
                    nc.vector.tensor_add(h32[:], nt_[:], zd2[:])
                    h16 = th.tile([H, NSEQ], dt.float16, tag="h16", name="h16")
                    nc.vector.tensor_copy(h16[:], h32[:])
                stg = tp.tile([H, 128], dt.float32)
                nc.vector.memset(stg[:], 0.0)
                nc.vector.tensor_copy(stg[:, 0:NSEQ], h32[:])
                id32 = tp.tile([128, 128], dt.float32)
                nc.sync.dma_start(id32[:], id32_d[:])
                stps = rps.tile([128, 128], dt.float32, tag="stps", name="stps",
                                bufs=1)
                nc.tensor.transpose(stps[:], stg[:], id32[:])
                stT = tp.tile([128, H], dt.float32)
                nc.vector.tensor_copy(stT[:], stps[:])
                nc.sync.dma_start(ag_in[:], stT[0:NSEQ, :])
            if not no_ag:
                nc.gpsimd.collective_compute(
                    "AllGather", mybir.AluOpType.bypass, replica_groups=RG,
                    ins=[ag_in[:]], outs=[ag_out[:]])

            if do_traj and not do_head:
                with tc.tile_pool(name="dummy2", bufs=1) as dp:
                    dz = dp.tile([B, 1], dt.float32)
                    nc.vector.memset(dz[:], 0.5)
                    nc.sync.dma_start(out_d[:], dz[:])
            # ---- head ----
            if do_traj and do_head:
              with (
                tc.tile_pool(name="head", bufs=1) as hd,
                tc.tile_pool(name="headps", bufs=1, space="PSUM") as hps,
            ):
                S = hd.tile([128, H], dt.float32)
                nc.sync.dma_start(S[:], ag_out[:])
                id32h = hd.tile([128, 128], dt.float32)
                nc.sync.dma_start(id32h[:], id32_d[:])
                STp = hps.tile([128, 128], dt.float32, tag="STp", name="STp")
                nc.tensor.transpose(STp[:], S[:], id32h[:])
                ST = hd.tile([H, 128], dt.float32)
                nc.vector.tensor_copy(ST[:], STp[:])
                on128 = hd.tile([H, 1], dt.float32)
                nc.vector.memset(on128[:], 1.0)
                on1 = hd.tile([1, H], dt.float32)
                nc.vector.memset(on1[:], 1.0)
                musum = hps.tile([1, B], dt.float32, tag="musum", name="musum")
                nc.tensor.matmul(musum[:], on128[:], ST[:, 0:B],
                                 start=True, stop=False)
                nc.tensor.matmul(musum[:], on128[:], ST[:, B:2 * B],
                                 start=False, stop=True)
                mur = hd.tile([1, B], dt.float32)
                nc.scalar.activation(mur[:], musum[:], AF.Copy, scale=1.0 / 256.0)
                MU = hps.tile([H, B], dt.float32, tag="MU", name="MU")
                nc.tensor.matmul(MU[:], on1[:], mur[:])
                Sc1 = hd.tile([H, B], dt.float32)
                nc.vector.tensor_sub(Sc1[:], ST[:, 0:B], MU[:])
                Sc2 = hd.tile([H, B], dt.float32)
                nc.vector.tensor_sub(Sc2[:], ST[:, B:2 * B], MU[:])
                q1 = hd.tile([H, B], dt.float32)
                nc.scalar.square(q1[:], Sc1[:])
                q2 = hd.tile([H, B], dt.float32)
                nc.scalar.square(q2[:], Sc2[:])
                ssq = hps.tile([1, B], dt.float32, tag="ssq", name="ssq")
                nc.tensor.matmul(ssq[:], on128[:], q1[:], start=True, stop=False)
                nc.tensor.matmul(ssq[:], on128[:], q2[:], start=False, stop=True)
                epsb = hd.tile([1, 1], dt.float32)
                nc.vector.memset(epsb[:], EPS)
                sd = hd.tile([1, B], dt.float32)
                nc.scalar.activation(sd[:], ssq[:], AF.Sqrt, scale=1.0 / 256.0,
                                     bias=epsb[:])
                rstd = hd.tile([1, B], dt.float32)
                nc.vector.reciprocal(rstd[:], sd[:])
                RSTDp = hps.tile([H, B], dt.float32, tag="RSTD", name="RSTDp")
                nc.tensor.matmul(RSTDp[:], on1[:], rstd[:])
                RSTD = hd.tile([H, B], dt.float32)
                nc.vector.tensor_copy(RSTD[:], RSTDp[:])
                w1a_t = hd.tile([H, H], dt.float32)
                nc.sync.dma_start(w1a_t[:], w1a[:])
                w1b_t = hd.tile([H, H], dt.float32)
                nc.sync.dma_start(w1b_t[:], w1b[:])
                P = hps.tile([H, B], dt.float32, tag="P", name="P")
                nc.tensor.matmul(P[:], w1a_t[:], Sc1[:], start=True, stop=False)
                nc.tensor.matmul(P[:], w1b_t[:], Sc2[:], start=False, stop=True)
                t1 = hd.tile([H, B], dt.float32)
                nc.vector.tensor_mul(t1[:], P[:], RSTD[:])
                w1d_t = hd.tile([DYN, H], dt.float32)
                nc.sync.dma_start(w1d_t[:], w1d[:])
                dyn_t = hd.tile([DYN, B], dt.float32)
                nc.sync.dma_start(dyn_t[:], dynT_d[:])
                Pd = hps.tile([H, B], dt.float32, tag="Pd", name="Pd")
                nc.tensor.matmul(Pd[:], w1d_t[:], dyn_t[:])
                t2 = hd.tile([H, B], dt.float32)
                nc.vector.tensor_add(t2[:], t1[:], Pd[:])
                c1_t = hd.tile([H, 1], dt.float32)
                nc.sync.dma_start(c1_t[:], c1_d[:])
                z1 = hd.tile([H, B], dt.float32)
                if plan.get("gelu_exact", True):
                    nc.scalar.activation(z1[:], t2[:], AF.Gelu, bias=c1_t[:])
                else:
                    u = hd.tile([H, B], dt.float32)
                    nc.scalar.activation(u[:], t2[:], AF.Identity, bias=c1_t[:])
                    sgm = hd.tile([H, B], dt.float32)
                    nc.scalar.activation(sgm[:], u[:], AF.Sigmoid, scale=1.702)
                    nc.vector.tensor_mul(z1[:], u[:], sgm[:])
                w2_t = hd.tile([H, 1], dt.float32)
                nc.sync.dma_start(w2_t[:], w2_d[:])
                ops = hps.tile([B, 1], dt.float32, tag="ops", name="ops")
                nc.tensor.matmul(ops[:], z1[:], w2_t[:])
                b2b = hd.tile([B, 1], dt.float32)
                nc.vector.memset(b2b[:], b2c)
                ores = hd.tile([B, 1], dt.float32)
                nc.scalar.activation(ores[:], ops[:], AF.Identity, bias=b2b[:])
                nc.sync.dma_start(out_d[:], ores[:])

    nc.compile()
    return nc


_last_results = None


def kernel(**inputs):
    global _last_results
    in_maps, plan, extras = _preprocess(**inputs)
    nc = _build(plan)
    from concourse.bass_utils import run_bass_kernel_spmd
    res = run_bass_kernel_spmd(nc, in_maps, core_ids=list(range(W)))
    _last_results = res
    return np.asarray(res.results[0]["out"], np.float32).reshape(B).copy()

